# revision 6
# baseline (speedup 1.0000x reference)
"""Trainium2 Bass kernel for an RNN-T JointNet (dense_mlp).

Reference computation (per batch b):
    joint[t,u,:] = enc[b,t,:] + pred[b,u,:]
    h            = tanh(joint @ w1.T + b1)
    logits       = h @ w2.T + b2          -> (B, T, U, V)

Key algebraic restructuring: fc1 distributes over the broadcast-add,
    fc1(joint)[t,u,h] = (enc[b,t] @ w1.T)[h] + (pred[b,u] @ w1.T)[h] + b1[h]
so the big (T*U, D) @ (D, H) matmul collapses to two tiny matmuls
(E1 = enc@w1.T, P1 = pred@w1.T) plus a broadcast add that is fused into
the tanh activation's per-partition bias. Only fc2 remains a full-size
matmul: (T*U, H) @ (H, V).

Sharding: data-parallel over batch B=8, one batch element per NeuronCore.
No collectives: each core computes its own (T, U, V) output slab.

All operands are staged host-side in transposed (contraction-major)
layout so the kernel needs no on-device transposes.
"""

import os
import sys

for _p in ("/opt/trn_rl_repo",):
    if _p not in sys.path:
        sys.path.insert(0, _p)

import numpy as np
import ml_dtypes
from contextlib import ExitStack

import concourse.bass as bass
import concourse.mybir as mybir
import concourse.tile as tile
from concourse import bacc
from concourse.bass_utils import run_bass_kernel_spmd

B, T, U, D, H, V = 8, 256, 64, 512, 512, 512
P = 128          # partitions
KD = D // P      # 4 contraction tiles for fc1
KH = H // P      # 4 contraction tiles for fc2
TB = T // P      # 2 t-blocks per core

F32 = mybir.dt.float32

# fc2 matmul operand dtype: "f32" (exact, 4 cyc/row), "f32r" (fast fp32
# replicated mode), "bf16" (fast, ~1e-3 rel err)
FC2_MODE = os.environ.get("JOINTNET_FC2_MODE", "f32r")


def _build_nc(fc2_mode: str) -> bass.Bass:
    nc = bacc.Bacc(trn_type="TRN2", target_bir_lowering=False, debug=False)

    if fc2_mode == "bf16":
        fc2_store_dt = mybir.dt.bfloat16
    elif fc2_mode == "f32r":
        fc2_store_dt = mybir.dt.float32r
    else:
        fc2_store_dt = F32

    enc_t = nc.dram_tensor("enc_t", [D, T], F32, kind="ExternalInput").ap()
    pred_t = nc.dram_tensor("pred_t", [D, U], F32, kind="ExternalInput").ap()
    w1_t = nc.dram_tensor("w1_t", [D, H], F32, kind="ExternalInput").ap()
    w2_t = nc.dram_tensor("w2_t", [H, V], fc2_store_dt, kind="ExternalInput").ap()
    b1_t = nc.dram_tensor("b1_t", [P, KH], F32, kind="ExternalInput").ap()
    b2_t = nc.dram_tensor("b2_t", [P, V], F32, kind="ExternalInput").ap()
    out = nc.dram_tensor("out", [T, U, V], F32, kind="ExternalOutput").ap()

    with tile.TileContext(nc) as tc, ExitStack() as ctx:
        consts = ctx.enter_context(tc.tile_pool(name="consts", bufs=1))
        psum_i = ctx.enter_context(tc.tile_pool(name="psum_i", bufs=2, space="PSUM"))
        ht_pool = ctx.enter_context(tc.tile_pool(name="ht", bufs=8))
        psum_o = ctx.enter_context(tc.tile_pool(name="psum_o", bufs=4, space="PSUM"))
        osb_pool = ctx.enter_context(tc.tile_pool(name="osb", bufs=4))

        # ---- load constants -------------------------------------------------
        w1t_sb = []
        for k in range(KD):
            t_ = consts.tile([P, H], F32, tag=f"w1t{k}")
            nc.gpsimd.dma_start(t_[:], w1_t[k * P:(k + 1) * P, :])
            w1t_sb.append(t_)
        w2t_sb = []
        for k in range(KH):
            t_ = consts.tile([P, V], fc2_store_dt, tag=f"w2t{k}")
            nc.gpsimd.dma_start(t_[:], w2_t[k * P:(k + 1) * P, :])
            w2t_sb.append(t_)
        enct_sb = []
        for k in range(KD):
            t_ = consts.tile([P, T], F32, tag=f"enct{k}")
            nc.gpsimd.dma_start(t_[:], enc_t[k * P:(k + 1) * P, :])
            enct_sb.append(t_)
        predt_sb = []
        for k in range(KD):
            t_ = consts.tile([P, U], F32, tag=f"predt{k}")
            nc.gpsimd.dma_start(t_[:], pred_t[k * P:(k + 1) * P, :])
            predt_sb.append(t_)
        b1_sb = consts.tile([P, KH], F32, tag="b1")
        nc.gpsimd.dma_start(b1_sb[:], b1_t[:])
        b2_sb = consts.tile([P, V], F32, tag="b2")
        nc.gpsimd.dma_start(b2_sb[:], b2_t[:])

        # ---- E1T = (w1 @ enc.T), P1T = (w1 @ pred.T) + b1 -------------------
        # E1T[h, t], stored as KH tiles of [128, T]; P1T[h, u] likewise.
        e1t_sb = []
        p1t_sb = []
        for hk in range(KH):
            pt = psum_i.tile([P, T], F32, tag="pt")
            for dk in range(KD):
                nc.tensor.matmul(
                    pt[:],
                    lhsT=w1t_sb[dk][:, hk * P:(hk + 1) * P],
                    rhs=enct_sb[dk][:],
                    start=(dk == 0),
                    stop=(dk == KD - 1),
                )
            e1 = consts.tile([P, T], F32, tag=f"e1t{hk}")
            nc.scalar.copy(e1[:], pt[:])
            e1t_sb.append(e1)

            pu = psum_i.tile([P, U], F32, tag="pu")
            for dk in range(KD):
                nc.tensor.matmul(
                    pu[:],
                    lhsT=w1t_sb[dk][:, hk * P:(hk + 1) * P],
                    rhs=predt_sb[dk][:],
                    start=(dk == 0),
                    stop=(dk == KD - 1),
                )
            p1 = consts.tile([P, U], F32, tag=f"p1t{hk}")
            nc.scalar.add(p1[:], pu[:], b1_sb[:, hk:hk + 1])
            p1t_sb.append(p1)

        # ---- main loop: logits[t_block, u, :] --------------------------------
        tanh = mybir.ActivationFunctionType.Tanh
        for u in range(U):
            for tb in range(TB):
                po = psum_o.tile([P, V], F32, tag="po")
                for hk in range(KH):
                    ht = ht_pool.tile([P, P], fc2_store_dt, tag="ht")
                    nc.scalar.activation(
                        ht[:],
                        e1t_sb[hk][:, tb * P:(tb + 1) * P],
                        tanh,
                        bias=p1t_sb[hk][:, u:u + 1],
                    )
                    nc.tensor.matmul(
                        po[:],
                        lhsT=ht[:],
                        rhs=w2t_sb[hk][:],
                        start=(hk == 0),
                        stop=(hk == KH - 1),
                    )
                osb = osb_pool.tile([P, V], F32, tag="osb")
                nc.vector.tensor_add(osb[:], po[:], b2_sb[:])
                nc.sync.dma_start(out[tb * P:(tb + 1) * P, u, :], osb[:])

    nc.compile()
    return nc


_NC_CACHE: dict = {}


def _get_nc(fc2_mode: str) -> bass.Bass:
    if fc2_mode not in _NC_CACHE:
        _NC_CACHE[fc2_mode] = _build_nc(fc2_mode)
    return _NC_CACHE[fc2_mode]


def _make_in_maps(padded_enc_output, padded_pred_output, w1, b1, w2, b2, fc2_mode):
    w1_t = np.ascontiguousarray(w1.T)                       # [D, H]
    w2_t = np.ascontiguousarray(w2.T)                       # [H, V]
    if fc2_mode == "bf16":
        w2_t = w2_t.astype(ml_dtypes.bfloat16)
    b1_t = np.ascontiguousarray(b1.reshape(KH, P).T)        # [P, KH]
    b2_t = np.ascontiguousarray(np.broadcast_to(b2, (P, V)))  # [P, V]
    in_maps = []
    for b in range(B):
        in_maps.append({
            "enc_t": np.ascontiguousarray(padded_enc_output[b].T),   # [D, T]
            "pred_t": np.ascontiguousarray(padded_pred_output[b].T),  # [D, U]
            "w1_t": w1_t,
            "w2_t": w2_t,
            "b1_t": b1_t,
            "b2_t": b2_t,
        })
    return in_maps


def run(inputs: dict, trace: bool = False, fc2_mode: str | None = None, **spmd_kwargs):
    """Run the kernel; returns (output, BassKernelResults)."""
    fc2_mode = fc2_mode or FC2_MODE
    nc = _get_nc(fc2_mode)
    in_maps = _make_in_maps(fc2_mode=fc2_mode, **inputs)
    res = run_bass_kernel_spmd(nc, in_maps, core_ids=list(range(B)),
                               trace=trace, **spmd_kwargs)
    outp = np.stack([res.results[b]["out"] for b in range(B)], axis=0)
    return outp, res


def kernel(**inputs) -> np.ndarray:
    outp, _ = run(inputs, trace=False)
    return outp


# revision 7
# speedup vs baseline: 1.2053x; 1.2053x over previous
"""Trainium2 Bass kernel for an RNN-T JointNet (dense_mlp).

Reference computation (per batch b):
    joint[t,u,:] = enc[b,t,:] + pred[b,u,:]
    h            = tanh(joint @ w1.T + b1)
    logits       = h @ w2.T + b2          -> (B, T, U, V)

Key algebraic restructuring: fc1 distributes over the broadcast-add,
    fc1(joint)[t,u,h] = (enc[b,t] @ w1.T)[h] + (pred[b,u] @ w1.T)[h] + b1[h]
so the big (T*U, D) @ (D, H) matmul collapses to two tiny matmuls
(E1 = enc@w1.T, P1 = pred@w1.T) plus a broadcast add that is fused into
the tanh activation's per-partition bias. Only fc2 remains a full-size
matmul: (T*U, H) @ (H, V).

Sharding: data-parallel over batch B=8, one batch element per NeuronCore.
No collectives: each core computes its own (T, U, V) output slab.

All operands are staged host-side in transposed (contraction-major)
layout so the kernel needs no on-device transposes.
"""

import os
import sys

for _p in ("/opt/trn_rl_repo",):
    if _p not in sys.path:
        sys.path.insert(0, _p)

import numpy as np
import ml_dtypes
from contextlib import ExitStack

import concourse.bass as bass
import concourse.mybir as mybir
import concourse.tile as tile
from concourse import bacc
from concourse.bass_utils import run_bass_kernel_spmd

B, T, U, D, H, V = 8, 256, 64, 512, 512, 512
P = 128          # partitions
KD = D // P      # 4 contraction tiles for fc1
KH = H // P      # 4 contraction tiles for fc2
TB = T // P      # 2 t-blocks per core

F32 = mybir.dt.float32

# fc2 matmul operand dtype: "f32" (exact, 4 cyc/row), "f32r" (fast fp32
# replicated mode), "bf16" (fast, ~1e-3 rel err)
FC2_MODE = os.environ.get("JOINTNET_FC2_MODE", "f32r")


def _build_nc(fc2_mode: str) -> bass.Bass:
    nc = bacc.Bacc(trn_type="TRN2", target_bir_lowering=False, debug=False)

    if fc2_mode == "bf16":
        fc2_store_dt = mybir.dt.bfloat16
    elif fc2_mode == "f32r":
        fc2_store_dt = mybir.dt.float32r
    else:
        fc2_store_dt = F32

    fc1_dt = F32 if fc2_mode == "f32" else mybir.dt.float32r
    enc_t = nc.dram_tensor("enc_t", [D, T], fc1_dt, kind="ExternalInput").ap()
    pred_t = nc.dram_tensor("pred_t", [D, U], fc1_dt, kind="ExternalInput").ap()
    w1_t = nc.dram_tensor("w1_t", [D, H], fc1_dt, kind="ExternalInput").ap()
    w2_t = nc.dram_tensor("w2_t", [H, V], fc2_store_dt, kind="ExternalInput").ap()
    b1_t = nc.dram_tensor("b1_t", [P, KH], F32, kind="ExternalInput").ap()
    b2_t = nc.dram_tensor("b2_t", [P, V], F32, kind="ExternalInput").ap()
    out = nc.dram_tensor("out", [T, U, V], F32, kind="ExternalOutput").ap()

    with tile.TileContext(nc) as tc, ExitStack() as ctx:
        consts = ctx.enter_context(tc.tile_pool(name="consts", bufs=1))
        psum_i = ctx.enter_context(tc.tile_pool(name="psum_i", bufs=2, space="PSUM"))
        ht_pool = ctx.enter_context(tc.tile_pool(name="ht", bufs=8))
        psum_o = ctx.enter_context(tc.tile_pool(name="psum_o", bufs=4, space="PSUM"))
        osb_pool = ctx.enter_context(tc.tile_pool(name="osb", bufs=4))

        # ---- load constants -------------------------------------------------
        w1t_sb = []
        for k in range(KD):
            t_ = consts.tile([P, H], fc1_dt, tag=f"w1t{k}")
            nc.gpsimd.dma_start(t_[:], w1_t[k * P:(k + 1) * P, :])
            w1t_sb.append(t_)
        w2t_sb = []
        for k in range(KH):
            t_ = consts.tile([P, V], fc2_store_dt, tag=f"w2t{k}")
            nc.gpsimd.dma_start(t_[:], w2_t[k * P:(k + 1) * P, :])
            w2t_sb.append(t_)
        enct_sb = []
        for k in range(KD):
            t_ = consts.tile([P, T], fc1_dt, tag=f"enct{k}")
            nc.gpsimd.dma_start(t_[:], enc_t[k * P:(k + 1) * P, :])
            enct_sb.append(t_)
        predt_sb = []
        for k in range(KD):
            t_ = consts.tile([P, U], fc1_dt, tag=f"predt{k}")
            nc.gpsimd.dma_start(t_[:], pred_t[k * P:(k + 1) * P, :])
            predt_sb.append(t_)
        b1_sb = consts.tile([P, KH], F32, tag="b1")
        nc.gpsimd.dma_start(b1_sb[:], b1_t[:])
        b2_sb = consts.tile([P, V], F32, tag="b2")
        nc.gpsimd.dma_start(b2_sb[:], b2_t[:])

        # ---- E1T = (w1 @ enc.T), P1T = (w1 @ pred.T) + b1 -------------------
        # E1T[h, t], stored as KH tiles of [128, T]; P1T[h, u] likewise.
        e1t_sb = []
        p1t_sb = []
        for hk in range(KH):
            pt = psum_i.tile([P, T], F32, tag="pt")
            for dk in range(KD):
                nc.tensor.matmul(
                    pt[:],
                    lhsT=w1t_sb[dk][:, hk * P:(hk + 1) * P],
                    rhs=enct_sb[dk][:],
                    start=(dk == 0),
                    stop=(dk == KD - 1),
                )
            e1 = consts.tile([P, T], F32, tag=f"e1t{hk}")
            nc.scalar.copy(e1[:], pt[:])
            e1t_sb.append(e1)

            pu = psum_i.tile([P, U], F32, tag="pu")
            for dk in range(KD):
                nc.tensor.matmul(
                    pu[:],
                    lhsT=w1t_sb[dk][:, hk * P:(hk + 1) * P],
                    rhs=predt_sb[dk][:],
                    start=(dk == 0),
                    stop=(dk == KD - 1),
                )
            p1 = consts.tile([P, U], F32, tag=f"p1t{hk}")
            nc.scalar.add(p1[:], pu[:], b1_sb[:, hk:hk + 1])
            p1t_sb.append(p1)

        # ---- main loop: logits[t_block, u, :] --------------------------------
        # One tanh per (u, hk) over the full T columns (amortizes the ACT
        # SBUF-access overhead); both t-block matmuls read slices of it.
        tanh = mybir.ActivationFunctionType.Tanh
        for u in range(U):
            hts = []
            for hk in range(KH):
                ht = ht_pool.tile([P, T], fc2_store_dt, tag="ht")
                nc.scalar.activation(
                    ht[:],
                    e1t_sb[hk][:],
                    tanh,
                    bias=p1t_sb[hk][:, u:u + 1],
                )
                hts.append(ht)
            for tb in range(TB):
                po = psum_o.tile([P, V], F32, tag="po")
                for hk in range(KH):
                    nc.tensor.matmul(
                        po[:],
                        lhsT=hts[hk][:, tb * P:(tb + 1) * P],
                        rhs=w2t_sb[hk][:],
                        start=(hk == 0),
                        stop=(hk == KH - 1),
                    )
                osb = osb_pool.tile([P, V], F32, tag="osb")
                nc.vector.tensor_add(osb[:], po[:], b2_sb[:])
                nc.sync.dma_start(out[tb * P:(tb + 1) * P, u, :], osb[:])

    nc.compile()
    return nc


_NC_CACHE: dict = {}


def _get_nc(fc2_mode: str) -> bass.Bass:
    if fc2_mode not in _NC_CACHE:
        _NC_CACHE[fc2_mode] = _build_nc(fc2_mode)
    return _NC_CACHE[fc2_mode]


def _make_in_maps(padded_enc_output, padded_pred_output, w1, b1, w2, b2, fc2_mode):
    w1_t = np.ascontiguousarray(w1.T)                       # [D, H]
    w2_t = np.ascontiguousarray(w2.T)                       # [H, V]
    if fc2_mode == "bf16":
        w2_t = w2_t.astype(ml_dtypes.bfloat16)
    b1_t = np.ascontiguousarray(b1.reshape(KH, P).T)        # [P, KH]
    b2_t = np.ascontiguousarray(np.broadcast_to(b2, (P, V)))  # [P, V]
    in_maps = []
    for b in range(B):
        in_maps.append({
            "enc_t": np.ascontiguousarray(padded_enc_output[b].T),   # [D, T]
            "pred_t": np.ascontiguousarray(padded_pred_output[b].T),  # [D, U]
            "w1_t": w1_t,
            "w2_t": w2_t,
            "b1_t": b1_t,
            "b2_t": b2_t,
        })
    return in_maps


def run(inputs: dict, trace: bool = False, fc2_mode: str | None = None, **spmd_kwargs):
    """Run the kernel; returns (output, BassKernelResults)."""
    fc2_mode = fc2_mode or FC2_MODE
    nc = _get_nc(fc2_mode)
    in_maps = _make_in_maps(fc2_mode=fc2_mode, **inputs)
    res = run_bass_kernel_spmd(nc, in_maps, core_ids=list(range(B)),
                               trace=trace, **spmd_kwargs)
    outp = np.stack([res.results[b]["out"] for b in range(B)], axis=0)
    return outp, res


def kernel(**inputs) -> np.ndarray:
    outp, _ = run(inputs, trace=False)
    return outp
